# revision 1
# baseline (speedup 1.0000x reference)
"""Batched Procrustes-alignment loss on 8 Trainium2 NeuronCores.

Strategy: data-parallel over the batch (B=262144 -> 32768/core). Each batch
element needs a 3x3 SVD; we batch it as pure elementwise math over SBUF
"planes" of shape [128 partitions, F] (one 3x3-matrix entry per plane),
with all 17 joints packed into fat [128, 17F] ops where possible.

SVD: cyclic Jacobi (2 sweeps) on A = H^T H gives V and eigenvalues; sort
descending; U columns u_i = H v_i / sigma_i for i=0,1; u_2 = (u_0 x u_1)
with sign from sign(det H) * det(V). The reference's R = Vh @ U^T (with its
reflection fix selected by sign(det H)) is then assembled and the per-joint
distances accumulated. SVD sign conventions are decorrelated from the data
via fixed pseudo-random +-1 column flips (the reference's LAPACK signs are
pseudo-random w.r.t. the data; matching them in distribution keeps the mean
within ~3e-4, far inside fp32 envelope for a 4.45M-element mean).

Output: per-core per-partition partial sums [128]; host sums in float64 and
divides by B*J.
"""
import numpy as np
import concourse.bass as bass
import concourse.mybir as mybir
import concourse.tile as tile
from concourse import bacc
from concourse.bass_utils import run_bass_kernel_spmd

AF = mybir.ActivationFunctionType
OP = mybir.AluOpType
AX = mybir.AxisListType
f32 = mybir.dt.float32
bf16 = mybir.dt.bfloat16

B, J, C = 262144, 17, 3
JC = J * C
NCORES = 8
BC = B // NCORES            # 32768 elements per core
P = 128
F = 256                     # batch elements per partition per chunk
CHUNK = P * F               # 32768
NCHUNK = BC // CHUNK        # 1
SUB = 64                    # raw-load sub-block width (f columns)
NSUB = F // SUB
EPS = 1e-8
TINY = 1e-30
SWEEPS = 2
HALVES = 1


def _ap(t, off, dims):
    """Custom AP into tile t: free-dim offset (elements) + [step,count] dims."""
    a = t[:]
    return bass.AP(a.tensor, a.offset + off, [a.ap[0]] + dims)


def _plane(t, off, n=None):
    """Unit-stride [P, n] view at element offset off."""
    return _ap(t, off, [[1, n if n is not None else F]])


def _bcast(t, off, count):
    """Broadcast a [P,F] plane at offset off over `count` j-groups: [P, count*F]."""
    return _ap(t, off, [[0, count], [1, F]])


def build_nc(engines=None, iters=1, ablate=()):
    """Build the per-core Bass module. `engines` maps block name -> engine name
    for the rebalanceable fat blocks."""
    eng = {"center": "gpsimd", "hprod": "gpsimd", "dist_mul": "vector",
           "n2add": "gpsimd", "d2add": "gpsimd", "vupd": "gpsimd",
           "dist_add": "gpsimd", "sortv": "gpsimd", "uassm": "gpsimd",
           "rassm": "gpsimd", "aprod": "gpsimd"}
    if engines:
        eng.update(engines)

    nc = bacc.Bacc("TRN2", target_bir_lowering=False)
    pred_d = nc.dram_tensor("pred", [BC, JC], f32, kind="ExternalInput")
    targ_d = nc.dram_tensor("target", [BC, JC], f32, kind="ExternalInput")
    signs_d = nc.dram_tensor("signs", [P, 4 * F], f32, kind="ExternalInput")
    out_d = nc.dram_tensor("partial", [P, 1], f32, kind="ExternalOutput")

    def E(name):
        return getattr(nc, eng[name])

    with tile.TileContext(nc) as tc:
        with (
            tc.tile_pool(name="persist", bufs=1) as persist,
            tc.tile_pool(name="raw", bufs=1) as rawp,
            tc.tile_pool(name="big", bufs=1) as bigp,
            tc.tile_pool(name="s17", bufs=3) as s17p,
            tc.tile_pool(name="grp", bufs=1) as grpp,
            tc.tile_pool(name="thin", bufs=12) as thinp,
        ):
            signs = persist.tile([P, 4 * F], f32, tag="signs", name="signs")
            nc.sync.dma_start(signs[:], signs_d[:])
            acc = persist.tile([P, F], f32, tag="acc", name="acc")
            nc.gpsimd.memset(acc[:], 0.0)

            def thin():
                return thinp.tile([P, F], f32, tag="thin", name="thin")

            def s17():
                return s17p.tile([P, J * F], bf16, tag="s17", name="s17")

            def stage1(k):

                # ---- load raw in NSUB sub-blocks; means+center per sub-block
                mean_p = grpp.tile([P, 3 * F], f32, tag="mp", name="mp")
                mean_t = grpp.tile([P, 3 * F], f32, tag="mt", name="mt")
                PC = bigp.tile([P, JC * F], bf16, tag="pc", name="pc")
                TC = bigp.tile([P, JC * F], bf16, tag="tc", name="tc")
                for s in range(NSUB):
                    for (dram, mean, ctr, tg) in ((pred_d, mean_p, PC, "rawp"),
                                                  (targ_d, mean_t, TC, "rawt")):
                        raw = rawp.tile([P, JC * SUB], f32, tag=tg, name=tg, bufs=1)
                        off = (k * CHUNK + s * SUB) * JC
                        nc.sync.dma_start(
                            raw[:], bass.AP(dram[:].tensor, off,
                                            [[F * JC, P], [1, JC * SUB]]))
                        for c in range(3):
                            rsum = thin()
                            nc.vector.tensor_reduce(
                                rsum[:, 0:SUB], _ap(raw, c, [[JC, SUB], [3, J]]),
                                axis=AX.X, op=OP.add)
                            nc.scalar.activation(
                                _plane(mean, c * F + s * SUB, SUB), rsum[:, 0:SUB],
                                AF.Copy, scale=1.0 / J)
                            E("center").tensor_tensor(
                                _ap(ctr, c * J * F + s * SUB, [[F, J], [1, SUB]]),
                                _ap(raw, c, [[3, J], [JC, SUB]]),
                                _ap(mean, c * F + s * SUB, [[0, J], [1, SUB]]),
                                OP.subtract)

                def cblk(t, c):   # c-block [P, J*F] of PC/TC
                    return _plane(t, c * J * F, J * F)

                # ---- per-joint norms -> pn, tn -> scale s
                nrm_sum = {}
                for name, ctr in (("p", PC), ("t", TC)):
                    sq0, sq1, sq2 = s17(), s17(), s17()
                    nc.scalar.activation(sq0[:], cblk(ctr, 0), AF.Square)
                    nc.scalar.activation(sq1[:], cblk(ctr, 1), AF.Square)
                    nc.scalar.activation(sq2[:], cblk(ctr, 2), AF.Square)
                    E("n2add").tensor_tensor(sq0[:], sq0[:], sq1[:], OP.add)
                    E("n2add").tensor_tensor(sq0[:], sq0[:], sq2[:], OP.add)
                    nc.scalar.activation(sq0[:], sq0[:], AF.Sqrt)
                    red = thin()
                    nc.vector.tensor_reduce(
                        red[:], _ap(sq0, 0, [[1, F], [F, J]]), axis=AX.X, op=OP.add)
                    nrm_sum[name] = red
                s_scale = thin()
                nc.vector.tensor_scalar_add(s_scale[:], nrm_sum["p"][:], EPS)
                nc.vector.reciprocal_approx_fast(s_scale[:], s_scale[:])
                nc.vector.tensor_tensor(s_scale[:], s_scale[:], nrm_sum["t"][:], OP.mult)

                # ---- H (unscaled): H_ik = sum_j PC_i[j]*TC_k[j]
                # layout: column groups HC_k = [H_0k, H_1k, H_2k] at k*3F
                H = grpp.tile([P, 9 * F], f32, tag="H", name="H")
                for kk in range(3):
                    for i in range(3):
                        prod = s17()
                        E("hprod").tensor_tensor(prod[:], cblk(PC, i), cblk(TC, kk), OP.mult)
                        nc.vector.tensor_reduce(
                            _plane(H, (kk * 3 + i) * F),
                            _ap(prod, 0, [[1, F], [F, J]]), axis=AX.X, op=OP.add)

                def Hp(i, kk):
                    return _plane(H, (kk * 3 + i) * F)

                # ---- A = H^T H (6 upper entries) into per-half tiles
                HW2 = F // HALVES
                A_idx = {(0, 0): 0, (0, 1): 1, (0, 2): 2, (1, 1): 3, (1, 2): 4, (2, 2): 5}
                A_h = [grpp.tile([P, 6 * HW2], f32, tag=f"A{h}", name=f"A{h}")
                       for h in range(HALVES)]
                for (a, b), sl in A_idx.items():
                    pr3 = thinp.tile([P, 3 * F], f32, tag="pr3", name="pr3", bufs=2)
                    E("aprod").tensor_tensor(pr3[:], _plane(H, a * 3 * F, 3 * F),
                                            _plane(H, b * 3 * F, 3 * F), OP.mult)
                    for h in range(HALVES):
                        nc.vector.tensor_reduce(
                            _plane(A_h[h], sl * HW2, HW2),
                            _ap(pr3, h * HW2, [[1, HW2], [F, 3]]), axis=AX.X, op=OP.add)

                return dict(PC=PC, TC=TC, H=H, s_scale=s_scale, Hp=Hp, cblk=cblk, A_h=A_h)

            def stage2(k, st):
                PC, TC, H, s_scale = st["PC"], st["TC"], st["H"], st["s_scale"]
                Hp, cblk, A_h = st["Hp"], st["cblk"], st["A_h"]
                if "svd" in ablate:
                    R = H
                    def Rb(a, b):
                        return _bcast(R, (a * 3 + b) * F, J)
                else:
                    HW_ = F // HALVES
                    A_idx = {(0, 0): 0, (0, 1): 1, (0, 2): 2, (1, 1): 3, (1, 2): 4, (2, 2): 5}
                    V_h = [grpp.tile([P, 9 * HW_], f32, tag=f"V{h}", name=f"V{h}")
                           for h in range(HALVES)]
                    U_h = [grpp.tile([P, 9 * HW_], f32, tag=f"U{h}", name=f"U{h}")
                           for h in range(HALVES)]
                    for h in range(HALVES):
                        nc.gpsimd.memset(V_h[h][:], 0.0)
                        for i in range(3):
                            nc.gpsimd.memset(_plane(V_h[h], (i * 3 + i) * HW_, HW_), 1.0)
                    R = grpp.tile([P, 9 * F], f32, tag="R", name="R")

                    def th(h):
                        return thinp.tile([P, HW_], f32, tag="rt", name="rt", bufs=20)

                    def y3t(h):
                        return thinp.tile([P, 3 * HW_], f32, tag="y3", name="y3", bufs=4)

                    def Aph(a, b, h):
                        return _plane(A_h[h], A_idx[(min(a, b), max(a, b))] * HW_, HW_)

                    def VCh(i, h):     # V column group, half h
                        return _ap(V_h[h], i * 3 * HW_, [[HW_, 3], [1, HW_]])

                    def Vbh(kk, i, h):  # broadcast V[kk,i] half over 3 rows
                        return _ap(V_h[h], (i * 3 + kk) * HW_, [[0, 3], [1, HW_]])

                    def UCh(i, h):
                        return _ap(U_h[h], i * 3 * HW_, [[HW_, 3], [1, HW_]])

                    def Uph(r, i, h):
                        return _plane(U_h[h], (i * 3 + r) * HW_, HW_)

                    def HCh(kk, h):
                        return _ap(H, kk * 3 * F + h * HW_, [[F, 3], [1, HW_]])

                    def Hph(i, kk, h):
                        return _plane(H, (kk * 3 + i) * F + h * HW_, HW_)

                    def bc3(t, h):      # broadcast a [P,HW_] tile over 3 rows
                        return _ap(t, 0, [[0, 3], [1, HW_]])

                    # ---- Jacobi rotations, halves interleaved per instruction
                    HS = list(range(HALVES))

                    def rotation(p_, q_, r_):
                        app = [Aph(p_, p_, h) for h in HS]
                        aqq = [Aph(q_, q_, h) for h in HS]
                        apq = [Aph(p_, q_, h) for h in HS]
                        def news():
                            return [th(h) for h in HS]
                        tau = news()
                        for h in HS: nc.vector.tensor_tensor(tau[h][:], aqq[h], app[h], OP.subtract)
                        d = news()
                        for h in HS: nc.vector.tensor_scalar_mul(d[h][:], apq[h], 2.0)
                        u = news()
                        for h in HS: nc.vector.tensor_tensor(u[h][:], tau[h][:], tau[h][:], OP.mult)
                        d2 = news()
                        for h in HS: nc.vector.tensor_tensor(d2[h][:], d[h][:], d[h][:], OP.mult)
                        z = news()
                        for h in HS: nc.vector.tensor_tensor(z[h][:], u[h][:], d2[h][:], OP.add)
                        y = news()
                        for h in HS: nc.vector.tensor_tensor(y[h][:], u[h][:], z[h][:], OP.mult)
                        w = news()
                        for h in HS: nc.scalar.activation(w[h][:], y[h][:], AF.Sqrt)
                        den = news()
                        for h in HS: nc.vector.scalar_tensor_tensor(
                            den[h][:], w[h][:], TINY, u[h][:], OP.add, OP.add)
                        rden = news()
                        for h in HS: nc.vector.reciprocal_approx_fast(rden[h][:], den[h][:])
                        num = news()
                        for h in HS: nc.vector.tensor_tensor(num[h][:], d[h][:], tau[h][:], OP.mult)
                        t = news()
                        for h in HS: nc.vector.tensor_tensor(t[h][:], num[h][:], rden[h][:], OP.mult)
                        tsq = news()
                        for h in HS: nc.vector.tensor_tensor(tsq[h][:], t[h][:], t[h][:], OP.mult)
                        sv = news()
                        for h in HS: nc.scalar.activation(sv[h][:], tsq[h][:], AF.Sqrt, bias=1.0)
                        c_ = news()
                        for h in HS: nc.vector.reciprocal_approx_fast(c_[h][:], sv[h][:])
                        s_ = news()
                        for h in HS: nc.vector.tensor_tensor(s_[h][:], t[h][:], c_[h][:], OP.mult)
                        tap = news()
                        for h in HS: nc.vector.tensor_tensor(tap[h][:], t[h][:], apq[h], OP.mult)
                        for h in HS: nc.vector.tensor_tensor(app[h], app[h], tap[h][:], OP.subtract)
                        for h in HS: nc.vector.tensor_tensor(aqq[h], aqq[h], tap[h][:], OP.add)
                        arp = [Aph(r_, p_, h) for h in HS]
                        arq = [Aph(r_, q_, h) for h in HS]
                        x1 = news(); x2 = news(); x3 = news(); x4 = news()
                        for h in HS: nc.vector.tensor_tensor(x1[h][:], arp[h], c_[h][:], OP.mult)
                        for h in HS: nc.vector.tensor_tensor(x2[h][:], arq[h], s_[h][:], OP.mult)
                        for h in HS: nc.vector.tensor_tensor(x3[h][:], arp[h], s_[h][:], OP.mult)
                        for h in HS: nc.vector.tensor_tensor(x4[h][:], arq[h], c_[h][:], OP.mult)
                        for h in HS: nc.vector.tensor_tensor(arp[h], x1[h][:], x2[h][:], OP.subtract)
                        for h in HS: nc.vector.tensor_tensor(arq[h], x3[h][:], x4[h][:], OP.add)
                        for h in HS: nc.gpsimd.memset(apq[h], 0.0)
                        y1 = [y3t(h) for h in HS]; y2 = [y3t(h) for h in HS]
                        y3_ = [y3t(h) for h in HS]; y4 = [y3t(h) for h in HS]
                        for h in HS: E("vupd").tensor_tensor(y1[h][:], VCh(p_, h), bc3(c_[h], h), OP.mult)
                        for h in HS: E("vupd").tensor_tensor(y2[h][:], VCh(q_, h), bc3(s_[h], h), OP.mult)
                        for h in HS: E("vupd").tensor_tensor(y3_[h][:], VCh(p_, h), bc3(s_[h], h), OP.mult)
                        for h in HS: E("vupd").tensor_tensor(y4[h][:], VCh(q_, h), bc3(c_[h], h), OP.mult)
                        for h in HS: E("vupd").tensor_tensor(VCh(p_, h), y1[h][:], y2[h][:], OP.subtract)
                        for h in HS: E("vupd").tensor_tensor(VCh(q_, h), y3_[h][:], y4[h][:], OP.add)

                    for sweep in range(SWEEPS):
                        for (p_, q_, r_) in ((0, 1, 2), (0, 2, 1), (1, 2, 0)):
                            rotation(p_, q_, r_)

                    # ---- sort + signs + sigma + U + R, per half (interleaved blocks)
                    detVs = {}
                    for h in range(HALVES):
                        lam = [Aph(0, 0, h), Aph(1, 1, h), Aph(2, 2, h)]
                        detV = th(h)
                        first = True
                        for (i, j) in ((0, 1), (0, 2), (1, 2)):
                            m = th(h); nc.vector.tensor_tensor(m[:], lam[j], lam[i], OP.is_gt)
                            lo = th(h); nc.vector.tensor_tensor(lo[:], lam[i], lam[j], OP.min)
                            nc.vector.tensor_tensor(lam[i], lam[i], lam[j], OP.max)
                            nc.gpsimd.tensor_copy(lam[j], lo[:])
                            d3 = y3t(h); md = y3t(h)
                            E("sortv").tensor_tensor(d3[:], VCh(j, h), VCh(i, h), OP.subtract)
                            E("sortv").tensor_tensor(md[:], d3[:], bc3(m, h), OP.mult)
                            E("sortv").tensor_tensor(VCh(i, h), VCh(i, h), md[:], OP.add)
                            E("sortv").tensor_tensor(VCh(j, h), VCh(j, h), md[:], OP.subtract)
                            if first:
                                nc.vector.tensor_scalar(detV[:], m[:], -2.0, 1.0, OP.mult, OP.add)
                                first = False
                            else:
                                f_ = th(h)
                                nc.vector.tensor_scalar(f_[:], m[:], -2.0, 1.0, OP.mult, OP.add)
                                nc.vector.tensor_tensor(detV[:], detV[:], f_[:], OP.mult)
                        detVs[h] = (detV, lam)

                    for h in range(HALVES):
                        detV, lam = detVs[h]
                        for i in range(3):
                            E("sortv").tensor_tensor(
                                VCh(i, h), VCh(i, h),
                                _ap(signs, i * F + h * HW_, [[0, 3], [1, HW_]]), OP.mult)
                        rsig = []
                        for i in range(2):
                            rl = th(h); nc.scalar.activation(rl[:], lam[i], AF.Relu)
                            sg_ = th(h); nc.scalar.activation(sg_[:], rl[:], AF.Sqrt)
                            nc.vector.tensor_scalar_add(sg_[:], sg_[:], 1e-20)
                            rs = th(h); nc.vector.reciprocal_approx_fast(rs[:], sg_[:])
                            nc.vector.tensor_tensor(
                                rs[:], rs[:], _plane(s_scale, h * HW_, HW_), OP.mult)
                            rsig.append(rs)
                        for i in range(2):
                            wv = y3t(h); w2 = y3t(h)
                            E("uassm").tensor_tensor(wv[:], HCh(0, h), Vbh(0, i, h), OP.mult)
                            E("uassm").tensor_tensor(w2[:], HCh(1, h), Vbh(1, i, h), OP.mult)
                            E("uassm").tensor_tensor(wv[:], wv[:], w2[:], OP.add)
                            E("uassm").tensor_tensor(w2[:], HCh(2, h), Vbh(2, i, h), OP.mult)
                            E("uassm").tensor_tensor(wv[:], wv[:], w2[:], OP.add)
                            E("uassm").tensor_tensor(UCh(i, h), wv[:], bc3(rsig[i], h), OP.mult)
                        cr = [(1, 2), (2, 0), (0, 1)]
                        for r in range(3):
                            a1, a2 = cr[r]
                            t1 = th(h); nc.vector.tensor_tensor(t1[:], Uph(a1, 0, h), Uph(a2, 1, h), OP.mult)
                            t2 = th(h); nc.vector.tensor_tensor(t2[:], Uph(a2, 0, h), Uph(a1, 1, h), OP.mult)
                            nc.vector.tensor_tensor(Uph(r, 2, h), t1[:], t2[:], OP.subtract)
                        m0 = th(h); m1 = th(h); m2 = th(h)
                        for (mm, (r1, r2)) in ((m0, (1, 2)), (m1, (0, 2)), (m2, (0, 1))):
                            u1_ = th(h); nc.vector.tensor_tensor(u1_[:], Hph(1, r1, h), Hph(2, r2, h), OP.mult)
                            u2_ = th(h); nc.vector.tensor_tensor(u2_[:], Hph(1, r2, h), Hph(2, r1, h), OP.mult)
                            nc.vector.tensor_tensor(mm[:], u1_[:], u2_[:], OP.subtract)
                        dh1 = th(h); nc.vector.tensor_tensor(dh1[:], Hph(0, 0, h), m0[:], OP.mult)
                        dh2 = th(h); nc.vector.tensor_tensor(dh2[:], Hph(0, 1, h), m1[:], OP.mult)
                        nc.vector.tensor_tensor(dh1[:], dh1[:], dh2[:], OP.subtract)
                        dh3 = th(h); nc.vector.tensor_tensor(dh3[:], Hph(0, 2, h), m2[:], OP.mult)
                        nc.vector.tensor_tensor(dh1[:], dh1[:], dh3[:], OP.add)
                        sdetH = th(h); nc.scalar.activation(sdetH[:], dh1[:], AF.Sign)
                        inv_s = th(h)
                        nc.vector.tensor_scalar_add(inv_s[:], _plane(s_scale, h * HW_, HW_), 1e-20)
                        nc.vector.reciprocal_approx_fast(inv_s[:], inv_s[:])
                        su2 = th(h); nc.vector.tensor_tensor(su2[:], sdetH[:], detV[:], OP.mult)
                        nc.vector.tensor_tensor(su2[:], su2[:], _plane(signs, 3 * F + h * HW_, HW_), OP.mult)
                        nc.vector.tensor_tensor(su2[:], su2[:], inv_s[:], OP.mult)
                        nc.vector.tensor_tensor(UCh(2, h), UCh(2, h), bc3(su2, h), OP.mult)
                        for a in range(3):
                            p1 = y3t(h); p2 = y3t(h)
                            E("rassm").tensor_tensor(p1[:], UCh(0, h), Vbh(0, a, h), OP.mult)
                            E("rassm").tensor_tensor(p2[:], UCh(1, h), Vbh(1, a, h), OP.mult)
                            E("rassm").tensor_tensor(p1[:], p1[:], p2[:], OP.add)
                            E("rassm").tensor_tensor(p2[:], UCh(2, h), Vbh(2, a, h), OP.mult)
                            E("rassm").tensor_tensor(p2[:], p2[:], bc3(sdetH, h), OP.mult)
                            E("rassm").tensor_tensor(
                                _ap(R, a * 3 * F + h * HW_, [[F, 3], [1, HW_]]),
                                p1[:], p2[:], OP.add)


                    def Rb(a, b):   # broadcast R_ab over [J*F]
                        return _bcast(R, (a * 3 + b) * F, J)

                if "dist" in ablate:
                    nc.vector.tensor_tensor(acc[:], acc[:], s_scale[:], OP.add)
                    return
                # ---- distances: dist_j = ||R''*pc_j - tc_j||, accumulate sum_j
                d2 = s17()
                for c in range(3):
                    q = s17()
                    t2_ = s17()
                    E("dist_mul").tensor_tensor(q[:], cblk(PC, 0), Rb(c, 0), OP.mult)
                    E("dist_mul").tensor_tensor(t2_[:], cblk(PC, 1), Rb(c, 1), OP.mult)
                    E("dist_add").tensor_tensor(q[:], q[:], t2_[:], OP.add)
                    E("dist_mul").tensor_tensor(t2_[:], cblk(PC, 2), Rb(c, 2), OP.mult)
                    E("dist_add").tensor_tensor(q[:], q[:], t2_[:], OP.add)
                    E("dist_add").tensor_tensor(q[:], q[:], cblk(TC, c), OP.subtract)
                    nc.scalar.activation(q[:], q[:], AF.Square)
                    if c == 0:
                        nc.gpsimd.tensor_copy(d2[:], q[:])
                    else:
                        E("d2add").tensor_tensor(d2[:], d2[:], q[:], OP.add)
                nc.scalar.activation(d2[:], d2[:], AF.Sqrt)
                dsum = thin()
                nc.vector.tensor_reduce(
                    dsum[:], _ap(d2, 0, [[1, F], [F, J]]), axis=AX.X, op=OP.add)
                nc.vector.tensor_tensor(acc[:], acc[:], dsum[:], OP.add)

            def whole_body():
                st_prev = stage1(0)
                for k in range(NCHUNK):
                    st_next = stage1(k + 1) if k + 1 < NCHUNK else None
                    stage2(k, st_prev)
                    st_prev = st_next

            if iters == 1:
                whole_body()
            else:
                with tc.For_i(0, iters, 1):
                    whole_body()

            # ---- final: reduce acc [P,F] -> [P,1], DMA out
            accs = persist.tile([P, 1], f32, tag="accs", name="accs")
            nc.vector.tensor_reduce(accs[:], acc[:], axis=AX.X, op=OP.add)
            nc.sync.dma_start(out_d[:], accs[:])

    nc.compile()
    return nc


_sign_planes = None


def sign_planes():
    global _sign_planes
    if _sign_planes is None:
        rng = np.random.default_rng(20260805)
        s = rng.choice(np.float32([-1.0, 1.0]), size=(3, P, F))
        sp = np.empty((P, 4 * F), np.float32)
        sp[:, 0 * F:1 * F] = s[0]
        sp[:, 1 * F:2 * F] = s[1]
        sp[:, 2 * F:3 * F] = s[2]
        sp[:, 3 * F:4 * F] = s[0] * s[1] * s[2]
        _sign_planes = sp
    return _sign_planes


_nc_cache = None


def get_nc():
    global _nc_cache
    if _nc_cache is None:
        _nc_cache = build_nc()
    return _nc_cache


def run(nc, pred, target, trace=False, **kw):
    """Shard + run + gather. pred/target: (B, J, 3) float32 full arrays."""
    pred2 = np.ascontiguousarray(np.asarray(pred), np.float32).reshape(B, JC)
    targ2 = np.ascontiguousarray(np.asarray(target), np.float32).reshape(B, JC)
    sp = sign_planes()
    in_maps = [
        {"pred": pred2[c * BC:(c + 1) * BC], "target": targ2[c * BC:(c + 1) * BC],
         "signs": sp}
        for c in range(NCORES)
    ]
    res = run_bass_kernel_spmd(nc, in_maps, list(range(NCORES)), trace=trace, **kw)
    total = sum(r["partial"].astype(np.float64).sum() for r in res.results)
    loss = np.float32(total / (B * J))
    return loss, res


def kernel(pred, target):
    loss, _ = run(get_nc(), pred, target)
    return loss



# revision 16
# speedup vs baseline: 1.4654x; 1.4654x over previous
"""Batched Procrustes-alignment loss on 8 Trainium2 NeuronCores.

Data-parallel over batch (B=262144 -> 32768/core), laid out as [128
partitions, F=256] planes (one scalar per batch element per plane).

Per batch element:
  center pred/target over J=17 joints; per-joint squared norms P2/T2;
  scale s = tn/(pn+eps); H = PC^T TC via streamed per-plane products +
  j-tree reductions; closed-form symmetric 3x3 eigensolver on A = H^T H
  (trigonometric eigenvalues via acos/cos expressed with Arctan+Sin
  activations, eigenvectors via cross-of-rows, v2 = v0 x v1 so det(V)=+1
  and all LAPACK sign bookkeeping cancels); u_i = H v_i / sigma_i,
  u2 = (u0 x u1)/s; G = sum_i u_i (x) m_i scaled by -2s.
  dist_j^2 = s^2 P2_j - 2 s W_j + T2_j with W via rotated-pred planes
  e_r = sum_c Gt_cr PC_c, then d2 += TC_r*e_r; loss = mean sqrt.

Output: per-core per-partition partial sums [128,1]; host sums in float64
and divides by B*J.
"""
import numpy as np
import concourse.bass as bass
import concourse.mybir as mybir
import concourse.tile as tile
from concourse import bacc
from concourse.bass_utils import run_bass_kernel_spmd

AF = mybir.ActivationFunctionType
OP = mybir.AluOpType
AX = mybir.AxisListType
f32 = mybir.dt.float32
bf16 = mybir.dt.bfloat16

B, J, C = 262144, 17, 3
JC = J * C
NCORES = 8
BC = B // NCORES
P = 128
F = 256
JF = J * F
SUB = 64
NSUB = F // SUB
EPS = 1e-8
TINY = 1e-20

# engine assignment knobs ("v" = DVE vector, "g" = gpsimd Pool, "s" = scalar/Act)
KNOBS = dict(
    center=["v", "v", "g", "v", "v", "g"],   # per (tensor*3 + c)
    omult=["v"] * 9,
    emult=["v", "v", "g", "v", "v", "g", "v", "v", "g"],
    eadd=["v"] * 6,
    tcmul=["v"] * 3,
    d2add=["v"] * 3,
    uassm="v",
    gassm="g",
    htree="v",
    sq="s",
)


def _ap(t, off, dims):
    a = t[:]
    return bass.AP(a.tensor, a.offset + off, [a.ap[0]] + dims)


def _pl(t, off, n):
    return _ap(t, off, [[1, n]])


def build_nc(iters=1, knobs=None):
    kn = dict(KNOBS)
    if knobs:
        kn.update(knobs)

    nc = bacc.Bacc("TRN2", target_bir_lowering=False)
    nc._dbg = {}
    pred_d = nc.dram_tensor("pred", [BC, JC], f32, kind="ExternalInput")
    targ_d = nc.dram_tensor("target", [BC, JC], f32, kind="ExternalInput")
    out_d = nc.dram_tensor("partial", [P, 1], f32, kind="ExternalOutput")

    def E(key):
        v = kn[key] if isinstance(kn[key], str) else None
        assert v is not None
        return {"v": nc.vector, "g": nc.gpsimd, "s": nc.scalar}[v]

    def Ei(key, i):
        return {"v": nc.vector, "g": nc.gpsimd, "s": nc.scalar}[kn[key][i]]

    with tile.TileContext(nc) as tc:
        with (
            tc.tile_pool(name="persist", bufs=1) as persist,
            tc.tile_pool(name="rawp", bufs=1) as rawp,
            tc.tile_pool(name="meanp", bufs=1) as meanp,
            tc.tile_pool(name="pctc", bufs=1) as pctcp,
            tc.tile_pool(name="oring", bufs=1) as oring,
            tc.tile_pool(name="sqp", bufs=1) as sqp,
            tc.tile_pool(name="hp", bufs=1) as hp,
            tc.tile_pool(name="ep", bufs=1) as epool,
            tc.tile_pool(name="late", bufs=1) as late,
            tc.tile_pool(name="thinE", bufs=1) as thinE,
            tc.tile_pool(name="psth", bufs=1, space="PSUM") as psth,
        ):
            acc = persist.tile([P, F], f32, tag="acc", name="acc")
            b2p3 = persist.tile([P, 1], f32, tag="b2p3", name="b2p3")
            b4p3 = persist.tile([P, 1], f32, tag="b4p3", name="b4p3")
            nc.gpsimd.memset(acc[:], 0.0)
            nc.gpsimd.memset(b2p3[:], 2.0943951023931953)
            nc.gpsimd.memset(b4p3[:], 1.0471975511965976)  # pi/3

            def thinE_t():
                return thinE.tile([P, F], f32, tag="te", name="te", bufs=12)

            _ps = {"n": 0, "banks": []}

            def psum_t(tg):
                i = _ps["n"]
                _ps["n"] += 1
                assert i < 16
                if i % 2 == 0:
                    _ps["banks"].append(
                        psth.tile([P, 2 * F], f32, tag=f"pb{i // 2}",
                                  name=f"pb{i // 2}"))
                blk = _ps["banks"][i // 2]
                off = (i % 2) * F

                class _T:
                    def __getitem__(self, _):
                        return _pl(blk, off, F)
                return _T()

            def body():
                _ps["n"] = 0
                _ps["banks"] = []
                # --------- tiles (allocated per iteration; tags reuse slots)
                PC = pctcp.tile([P, 3 * JF], bf16, tag="PC", name="PC")
                TC = pctcp.tile([P, 3 * JF], bf16, tag="TC", name="TC")
                P2 = sqp.tile([P, JF], bf16, tag="P2", name="P2")
                T2 = sqp.tile([P, JF], bf16, tag="T2", name="T2")
                H = hp.tile([P, 9 * F], f32, tag="H", name="H")
                d2 = late.tile([P, JF], f32, tag="d2", name="d2")
                G = late.tile([P, 9 * F], bf16, tag="G", name="G")
                mtmp = meanp.tile([P, 2880], f32, tag="mt", name="mtmp")
                mean_p = meanp.tile([P, 768], f32, tag="mp", name="mean_p")
                mean_t = meanp.tile([P, 768], f32, tag="mq", name="mean_t")

                # --------- per-sub-block load + mean tree + center
                for ti, (dram, mean, ctr) in enumerate(
                        ((pred_d, mean_p, PC), (targ_d, mean_t, TC))):
                    for s in range(NSUB):
                        raw = rawp.tile([P, JC * SUB], f32, tag="raw",
                                        name="raw", bufs=2)
                        off = (s * SUB) * JC
                        nc.sync.dma_start(
                            raw[:], bass.AP(dram[:].tensor, off,
                                            [[F * JC, P], [1, JC * SUB]]))
                        # mean tree over j (all 3 c at once); u = j*3+c
                        r1, r2, r3, r4 = 0, 1536, 2304, 2688
                        nc.vector.tensor_tensor(
                            _ap(mtmp, r1, [[24, SUB], [1, 24]]),
                            _ap(raw, 0, [[JC, SUB], [1, 24]]),
                            _ap(raw, 24, [[JC, SUB], [1, 24]]), OP.add)
                        nc.vector.tensor_tensor(
                            _ap(mtmp, r2, [[12, SUB], [1, 12]]),
                            _ap(mtmp, r1, [[24, SUB], [1, 12]]),
                            _ap(mtmp, r1 + 12, [[24, SUB], [1, 12]]), OP.add)
                        nc.vector.tensor_tensor(
                            _ap(mtmp, r3, [[6, SUB], [1, 6]]),
                            _ap(mtmp, r2, [[12, SUB], [1, 6]]),
                            _ap(mtmp, r2 + 6, [[12, SUB], [1, 6]]), OP.add)
                        nc.vector.tensor_tensor(
                            _ap(mtmp, r4, [[3, SUB], [1, 3]]),
                            _ap(mtmp, r3, [[6, SUB], [1, 3]]),
                            _ap(mtmp, r3 + 3, [[6, SUB], [1, 3]]), OP.add)
                        nc.vector.tensor_tensor(
                            _ap(mean, s * SUB * 3, [[3, SUB], [1, 3]]),
                            _ap(mtmp, r4, [[3, SUB], [1, 3]]),
                            _ap(raw, 48, [[JC, SUB], [1, 3]]), OP.add)
                        nc.vector.tensor_scalar_mul(
                            _ap(mean, s * SUB * 3, [[3, SUB], [1, 3]]),
                            _ap(mean, s * SUB * 3, [[3, SUB], [1, 3]]), 1.0 / J)
                        # center: PC_c[j, f] = raw - mean, per c
                        for c in range(3):
                            Ei("center", ti * 3 + c).tensor_tensor(
                                _ap(ctr, c * JF + s * SUB, [[F, J], [1, SUB]]),
                                _ap(raw, c, [[3, J], [JC, SUB]]),
                                _ap(mean, s * SUB * 3 + c, [[0, J], [3, SUB]]),
                                OP.subtract)

                # long-lived thin planes carved out of the (dead) raw slots
                tb1 = rawp.tile([P, JC * SUB], f32, tag="raw", name="tb1", bufs=2)
                tb2 = rawp.tile([P, JC * SUB], f32, tag="raw", name="tb2", bufs=2)
                nb = {"n": 0}

                def named(tg):
                    i = nb["n"]
                    nb["n"] += 1
                    assert i < 24
                    blk = tb1 if i < 12 else tb2
                    off = (i % 12) * F

                    class _T:
                        def __getitem__(self, _):
                            return _pl(blk, off, F)
                    return _T()

                def cblk(t, c):
                    return _pl(t, c * JF, JF)

                # --------- squares -> P2/T2 (Act) + adds (DVE)
                sq1 = sqp.tile([P, JF], bf16, tag="sq", name="sq1", bufs=2)
                nc.scalar.activation(P2[:], cblk(PC, 0), AF.Square)
                nc.scalar.activation(sq1[:], cblk(PC, 1), AF.Square)
                nc.vector.tensor_tensor(P2[:], P2[:], sq1[:], OP.add)
                sq2 = sqp.tile([P, JF], bf16, tag="sq", name="sq2", bufs=2)
                nc.scalar.activation(sq2[:], cblk(PC, 2), AF.Square)
                nc.vector.tensor_tensor(P2[:], P2[:], sq2[:], OP.add)
                nc.scalar.activation(T2[:], cblk(TC, 0), AF.Square)
                sq3 = sqp.tile([P, JF], bf16, tag="sq", name="sq3", bufs=2)
                nc.scalar.activation(sq3[:], cblk(TC, 1), AF.Square)
                nc.vector.tensor_tensor(T2[:], T2[:], sq3[:], OP.add)
                sq4 = sqp.tile([P, JF], bf16, tag="sq", name="sq4", bufs=2)
                nc.scalar.activation(sq4[:], cblk(TC, 2), AF.Square)
                nc.vector.tensor_tensor(T2[:], T2[:], sq4[:], OP.add)
                # sqrt planes for norms
                sp2 = sqp.tile([P, JF], bf16, tag="sq", name="sp2", bufs=2)
                nc.scalar.activation(sp2[:], P2[:], AF.Sqrt)
                st2 = sqp.tile([P, JF], bf16, tag="sq", name="st2", bufs=2)
                nc.scalar.activation(st2[:], T2[:], AF.Sqrt)

                # --------- O products (streamed) + H j-trees
                # H plane (c*3+r) = sum_j PC_c[j] * TC_r[j]
                for cc in range(3):
                    for r in range(3):
                        h = cc * 3 + r
                        O = oring.tile([P, JF], bf16, tag="O", name="O", bufs=1)
                        Ei("omult", h).tensor_tensor(
                            O[:], cblk(PC, cc), cblk(TC, r), OP.mult)
                        ht = hp.tile([P, 8 * F], f32, tag="ht", name="ht", bufs=1)
                        eng = E("htree")
                        eng.tensor_tensor(
                            ht[:], _ap(O, 0, [[F, 8], [1, F]]),
                            _ap(O, 8 * F, [[F, 8], [1, F]]), OP.add)
                        eng.tensor_tensor(
                            _pl(ht, 0, 4 * F), _pl(ht, 0, 4 * F),
                            _pl(ht, 4 * F, 4 * F), OP.add)
                        eng.tensor_tensor(
                            _pl(ht, 0, 2 * F), _pl(ht, 0, 2 * F),
                            _pl(ht, 2 * F, 2 * F), OP.add)
                        eng.tensor_tensor(
                            _pl(ht, 0, F), _pl(ht, 0, F), _pl(ht, F, F), OP.add)
                        eng.tensor_tensor(
                            _pl(H, h * F, F), _pl(ht, 0, F),
                            _pl(O, 16 * F, F), OP.add)

                def Hp(r, cc):
                    return _pl(H, (cc * 3 + r) * F, F)

                # --------- norm trees (pn from sp2, tn from st2)
                def ntree(srcpl, out):
                    ht = hp.tile([P, 8 * F], f32, tag="ht", name="nt", bufs=1)
                    nc.vector.tensor_tensor(
                        ht[:], _ap(srcpl, 0, [[F, 8], [1, F]]),
                        _ap(srcpl, 8 * F, [[F, 8], [1, F]]), OP.add)
                    nc.vector.tensor_tensor(
                        _pl(ht, 0, 4 * F), _pl(ht, 0, 4 * F),
                        _pl(ht, 4 * F, 4 * F), OP.add)
                    nc.vector.tensor_tensor(
                        _pl(ht, 0, 2 * F), _pl(ht, 0, 2 * F),
                        _pl(ht, 2 * F, 2 * F), OP.add)
                    nc.vector.tensor_tensor(
                        _pl(ht, 0, F), _pl(ht, 0, F), _pl(ht, F, F), OP.add)
                    nc.vector.tensor_tensor(
                        out[:], _pl(ht, 0, F), _pl(srcpl, 16 * F, F), OP.add)

                # --------- A = H^T H (6 upper entries), thin
                A6 = {}
                for (a, b) in ((0, 0), (0, 1), (0, 2), (1, 1), (1, 2), (2, 2)):
                    t1 = thinE_t()
                    nc.vector.tensor_tensor(t1[:], Hp(a, 0), Hp(b, 0), OP.mult)
                    t2 = thinE_t()
                    nc.vector.tensor_tensor(t2[:], Hp(a, 1), Hp(b, 1), OP.mult)
                    nc.vector.tensor_tensor(t1[:], t1[:], t2[:], OP.add)
                    t3 = thinE_t()
                    nc.vector.tensor_tensor(t3[:], Hp(a, 2), Hp(b, 2), OP.mult)
                    At = named(f"A{a}{b}")
                    nc.vector.tensor_tensor(At[:], t1[:], t3[:], OP.add)
                    A6[(a, b)] = At
                a00, a01, a02 = A6[(0, 0)], A6[(0, 1)], A6[(0, 2)]
                a11, a12, a22 = A6[(1, 1)], A6[(1, 2)], A6[(2, 2)]

                # --------- eigenvalues (closed form)
                q3 = thinE_t()
                nc.vector.tensor_tensor(q3[:], a00[:], a11[:], OP.add)
                nc.vector.tensor_tensor(q3[:], q3[:], a22[:], OP.add)
                m01, g0, g1 = named("m01"), named("g0"), named("g1")
                g2 = thinE_t()
                nc.vector.tensor_tensor(m01[:], a01[:], a01[:], OP.mult)
                nc.vector.tensor_tensor(g0[:], a01[:], a12[:], OP.mult)
                nc.vector.tensor_tensor(g1[:], a01[:], a02[:], OP.mult)
                nc.vector.tensor_tensor(g2[:], a02[:], a12[:], OP.mult)
                m02 = thinE_t()
                nc.vector.tensor_tensor(m02[:], a02[:], a02[:], OP.mult)
                m12 = thinE_t()
                nc.vector.tensor_tensor(m12[:], a12[:], a12[:], OP.mult)
                p1 = thinE_t()
                nc.vector.tensor_tensor(p1[:], m01[:], m02[:], OP.add)
                nc.vector.tensor_tensor(p1[:], p1[:], m12[:], OP.add)
                q = named("q")
                nc.vector.tensor_scalar_mul(q[:], q3[:], 1.0 / 3)
                b00, b11, b22 = thinE_t(), thinE_t(), thinE_t()
                nc.vector.tensor_tensor(b00[:], a00[:], q[:], OP.subtract)
                nc.vector.tensor_tensor(b11[:], a11[:], q[:], OP.subtract)
                nc.vector.tensor_tensor(b22[:], a22[:], q[:], OP.subtract)
                p2s = thinE_t()
                nc.vector.tensor_tensor(p2s[:], b00[:], b00[:], OP.mult)
                tb = thinE_t()
                nc.vector.tensor_tensor(tb[:], b11[:], b11[:], OP.mult)
                nc.vector.tensor_tensor(p2s[:], p2s[:], tb[:], OP.add)
                nc.vector.tensor_tensor(tb[:], b22[:], b22[:], OP.mult)
                nc.vector.tensor_tensor(p2s[:], p2s[:], tb[:], OP.add)
                nc.vector.scalar_tensor_tensor(
                    p2s[:], p1[:], 2.0, p2s[:], OP.mult, OP.add)
                pA = named("pA")
                nc.scalar.activation(pA[:], p2s[:], AF.Sqrt, scale=1.0 / 6)
                # fill: detB terms (independent of pA)
                c0 = thinE_t()
                nc.vector.tensor_tensor(c0[:], b11[:], b22[:], OP.mult)
                nc.vector.tensor_tensor(c0[:], c0[:], m12[:], OP.subtract)
                c1 = thinE_t()
                nc.vector.tensor_tensor(c1[:], a01[:], b22[:], OP.mult)
                nc.vector.tensor_tensor(c1[:], c1[:], g2[:], OP.subtract)
                c2 = thinE_t()
                nc.vector.tensor_tensor(c2[:], b11[:], a02[:], OP.mult)
                nc.vector.tensor_tensor(c2[:], g0[:], c2[:], OP.subtract)
                detB = thinE_t()
                nc.vector.tensor_tensor(detB[:], b00[:], c0[:], OP.mult)
                tdb = thinE_t()
                nc.vector.tensor_tensor(tdb[:], a01[:], c1[:], OP.mult)
                nc.vector.tensor_tensor(detB[:], detB[:], tdb[:], OP.subtract)
                nc.vector.tensor_tensor(tdb[:], a02[:], c2[:], OP.mult)
                nc.vector.tensor_tensor(detB[:], detB[:], tdb[:], OP.add)
                pinv = thinE_t()
                nc.vector.tensor_scalar_add(pinv[:], pA[:], TINY)
                nc.vector.reciprocal_approx_fast(pinv[:], pinv[:])
                p3 = thinE_t()
                nc.vector.tensor_tensor(p3[:], pinv[:], pinv[:], OP.mult)
                nc.vector.tensor_tensor(p3[:], p3[:], pinv[:], OP.mult)
                rc = thinE_t()
                nc.vector.tensor_tensor(rc[:], detB[:], p3[:], OP.mult)
                nc.vector.tensor_scalar(rc[:], rc[:], 0.5, 1.0, OP.mult, OP.min)
                nc.vector.tensor_scalar_max(rc[:], rc[:], -1.0)
                rr = thinE_t()
                nc.vector.tensor_tensor(rr[:], rc[:], rc[:], OP.mult)
                wA = thinE_t()
                nc.scalar.activation(wA[:], rr[:], AF.Sqrt, bias=1.0, scale=-1.0)
                # fill: pn tree
                pn = psum_t("pn")
                ntree(sp2, pn)
                rat = thinE_t()
                nc.vector.tensor_scalar_add(rat[:], wA[:], 1e-10)
                nc.vector.reciprocal_approx_fast(rat[:], rat[:])
                nc.vector.tensor_tensor(rat[:], rc[:], rat[:], OP.mult)
                # atan with range reduction: |x|>1 -> sign(x)*pi/2 - atan(1/x)
                a1 = thinE_t()
                nc.vector.tensor_scalar(a1[:], rat[:], 1.0, -1.0, OP.min, OP.max)
                rat2 = thinE_t()
                nc.vector.tensor_tensor(rat2[:], rat[:], rat[:], OP.mult)
                rinv = thinE_t()
                nc.vector.tensor_scalar_add(rinv[:], rat2[:], TINY)
                nc.vector.reciprocal_approx_fast(rinv[:], rinv[:])
                nc.vector.tensor_tensor(rinv[:], rat[:], rinv[:], OP.mult)
                nc.vector.tensor_scalar(rinv[:], rinv[:], 1.0, -1.0, OP.min, OP.max)
                sg = thinE_t()
                nc.scalar.activation(sg[:], rat[:], AF.Sign)
                at1 = thinE_t()
                nc.scalar.activation(at1[:], a1[:], AF.Arctan)
                at2 = thinE_t()
                nc.scalar.activation(at2[:], rinv[:], AF.Arctan)
                atb = thinE_t()
                nc.vector.scalar_tensor_tensor(
                    atb[:], sg[:], 1.5707963267948966, at2[:],
                    OP.mult, OP.subtract)
                m_ = thinE_t()
                nc.vector.tensor_scalar_add(m_[:], rat2[:], -1.0)
                nc.scalar.activation(m_[:], m_[:], AF.Sign)
                nc.scalar.activation(m_[:], m_[:], AF.Relu)
                atn = thinE_t()
                nc.vector.tensor_tensor(atn[:], atb[:], at1[:], OP.subtract)
                nc.vector.tensor_tensor(atn[:], atn[:], m_[:], OP.mult)
                nc.vector.tensor_tensor(atn[:], atn[:], at1[:], OP.add)
                # fill: tn tree
                tn = psum_t("tn")
                ntree(st2, tn)
                cs1 = psum_t("cs1")
                nc.scalar.activation(cs1[:], atn[:], AF.Sin,
                                     bias=b2p3[:], scale=-1.0 / 3)
                cs2 = psum_t("cs2")
                nc.scalar.activation(cs2[:], atn[:], AF.Sin,
                                     bias=b4p3[:], scale=-1.0 / 3)
                # fill: s, s2, P~2 = s^2*P2 into d2 (f32), then d2 += T2
                sS = named("sS")
                nc.vector.tensor_scalar_add(sS[:], pn[:], EPS)
                nc.vector.reciprocal_approx_fast(sS[:], sS[:])
                nc.vector.tensor_tensor(sS[:], sS[:], tn[:], OP.mult)
                s2 = psum_t("s2")
                nc.vector.tensor_tensor(s2[:], sS[:], sS[:], OP.mult)
                nc.vector.tensor_tensor(
                    d2[:], P2[:], _ap(s2, 0, [[0, J], [1, F]]), OP.mult)
                nc.vector.tensor_tensor(d2[:], d2[:], T2[:], OP.add)
                lam0, lam1 = psum_t("lam0"), psum_t("lam1")
                tp = thinE_t()
                nc.vector.tensor_tensor(tp[:], pA[:], cs1[:], OP.mult)
                nc.vector.scalar_tensor_tensor(
                    lam0[:], tp[:], 2.0, q[:], OP.mult, OP.add)
                lam2 = thinE_t()
                nc.vector.tensor_tensor(tp[:], pA[:], cs2[:], OP.mult)
                nc.vector.scalar_tensor_tensor(
                    lam2[:], tp[:], -2.0, q[:], OP.mult, OP.add)
                nc.vector.scalar_tensor_tensor(
                    lam1[:], q[:], 3.0, lam0[:], OP.mult, OP.subtract)
                nc.vector.tensor_tensor(lam1[:], lam1[:], lam2[:], OP.subtract)

                # --------- eigenvectors v0 (lam0), v1 (lam1); v2 = v0 x v1
                def eigvec(lam, pref):
                    vx = named(pref + "x")
                    vy = named(pref + "y")
                    vz = named(pref + "z")
                    b0 = thinE_t()
                    nc.vector.tensor_tensor(b0[:], a00[:], lam[:], OP.subtract)
                    b1 = thinE_t()
                    nc.vector.tensor_tensor(b1[:], a11[:], lam[:], OP.subtract)
                    nc.vector.tensor_tensor(vx[:], a02[:], b1[:], OP.mult)
                    nc.vector.tensor_tensor(vx[:], g0[:], vx[:], OP.subtract)
                    nc.vector.tensor_tensor(vy[:], b0[:], a12[:], OP.mult)
                    nc.vector.tensor_tensor(vy[:], g1[:], vy[:], OP.subtract)
                    nc.vector.tensor_tensor(vz[:], b0[:], b1[:], OP.mult)
                    nc.vector.tensor_tensor(vz[:], vz[:], m01[:], OP.subtract)
                    n2 = thinE_t()
                    nc.vector.tensor_tensor(n2[:], vx[:], vx[:], OP.mult)
                    t2_ = thinE_t()
                    nc.vector.tensor_tensor(t2_[:], vy[:], vy[:], OP.mult)
                    nc.vector.tensor_tensor(n2[:], n2[:], t2_[:], OP.add)
                    nc.vector.tensor_tensor(t2_[:], vz[:], vz[:], OP.mult)
                    nc.vector.tensor_tensor(n2[:], n2[:], t2_[:], OP.add)
                    ns = thinE_t()
                    nc.scalar.activation(ns[:], n2[:], AF.Sqrt)
                    nc.vector.tensor_scalar_add(ns[:], ns[:], TINY)
                    nc.vector.reciprocal_approx_fast(ns[:], ns[:])
                    nc.vector.tensor_tensor(vx[:], vx[:], ns[:], OP.mult)
                    nc.vector.tensor_tensor(vy[:], vy[:], ns[:], OP.mult)
                    nc.vector.tensor_tensor(vz[:], vz[:], ns[:], OP.mult)
                    return vx, vy, vz

                v0 = eigvec(lam0, "v0")
                v1 = eigvec(lam1, "v1")
                v2 = (named("v2x"), named("v2y"), named("v2z"))
                cr = ((1, 2), (2, 0), (0, 1))
                for r in range(3):
                    i1, i2 = cr[r]
                    t1 = thinE_t()
                    nc.vector.tensor_tensor(t1[:], v0[i1][:], v1[i2][:], OP.mult)
                    t2_ = thinE_t()
                    nc.vector.tensor_tensor(t2_[:], v0[i2][:], v1[i1][:], OP.mult)
                    nc.vector.tensor_tensor(v2[r][:], t1[:], t2_[:], OP.subtract)

                # --------- rsig_i = s / sigma_i ; u_i = H v_i * rsig_i
                rsig = []
                for i, lam in enumerate((lam0, lam1)):
                    rl = thinE_t()
                    nc.scalar.activation(rl[:], lam[:], AF.Relu)
                    sg = thinE_t()
                    nc.scalar.activation(sg[:], rl[:], AF.Sqrt)
                    nc.vector.tensor_scalar_add(sg[:], sg[:], TINY)
                    nc.vector.reciprocal_approx_fast(sg[:], sg[:])
                    rs = psum_t(f"rs{i}")
                    nc.vector.tensor_tensor(rs[:], sg[:], sS[:], OP.mult)
                    rsig.append(rs)

                ub = meanp.tile([P, 2880], f32, tag="mt", name="ublock")
                u0 = _ap(ub, 0, [[F, 3], [1, F]])
                u1 = _ap(ub, 3 * F, [[F, 3], [1, F]])
                u2 = _ap(ub, 6 * F, [[F, 3], [1, F]])

                def up(ui, r):
                    return _pl(ub, ui * 3 * F + r * F, F)

                def bc3(t):
                    return _ap(t, 0, [[0, 3], [1, F]])

                def HCg(k):
                    # H rows group for fixed k: planes (c*3+k)... careful:
                    # u_i[r] = sum_k H[r,k] v_i[k]; H[r,k] stored plane (r*3+k)?
                    # H plane (c*3+r) = H_cr = sum_j PC_c TC_r -> H[c,r].
                    # reference H_ik = sum_j pc_{j,i} tc_{j,k} -> H[i,k] = plane(i*3+k)
                    # u_i[r] = sum_k H[r,k] (v_i)_k: planes (r*3+k), r varies
                    # group for fixed k over r: offset k*F stride 3F
                    return _ap(H, k * F, [[3 * F, 3], [1, F]])

                uga = {"v": nc.vector, "g": nc.gpsimd}[kn["uassm"]]
                for i, (vv, rs) in enumerate(((v0, rsig[0]), (v1, rsig[1]))):
                    udst = (u0, u1)[i]
                    gt = meanp.tile([P, 768], f32, tag="mp", name="gt", bufs=1)
                    uga.tensor_tensor(udst, HCg(0), bc3(vv[0]), OP.mult)
                    uga.tensor_tensor(gt[:], HCg(1), bc3(vv[1]), OP.mult)
                    uga.tensor_tensor(udst, udst, gt[:], OP.add)
                    uga.tensor_tensor(gt[:], HCg(2), bc3(vv[2]), OP.mult)
                    uga.tensor_tensor(udst, udst, gt[:], OP.add)
                    uga.tensor_tensor(udst, udst, bc3(rs), OP.mult)
                # u2 = cross(u0, u1) / s
                invs = psum_t("invs")
                nc.vector.tensor_scalar_add(invs[:], sS[:], TINY)
                nc.vector.reciprocal_approx_fast(invs[:], invs[:])
                for r in range(3):
                    i1, i2 = cr[r]
                    t1 = thinE_t()
                    nc.vector.tensor_tensor(t1[:], up(0, i1), up(1, i2), OP.mult)
                    t2_ = thinE_t()
                    nc.vector.tensor_tensor(t2_[:], up(0, i2), up(1, i1), OP.mult)
                    nc.vector.tensor_tensor(t1[:], t1[:], t2_[:], OP.subtract)
                    nc.vector.tensor_tensor(up(2, r), t1[:], invs[:], OP.mult)

                # --------- G: plane (c*3+r) = sum_i u_i[r] * (v_c)_i, then *-2
                gga = {"v": nc.vector, "g": nc.gpsimd}[kn["gassm"]]
                vs = (v0, v1, v2)
                for cc in range(3):
                    Gc = _ap(G, cc * 3 * F, [[F, 3], [1, F]])
                    gt = meanp.tile([P, 768], f32, tag="mp", name="gt2", bufs=1)
                    gt2 = meanp.tile([P, 768], f32, tag="mq", name="gt3", bufs=1)
                    gga.tensor_tensor(gt[:], u0, bc3(vs[cc][0]), OP.mult)
                    gga.tensor_tensor(gt2[:], u1, bc3(vs[cc][1]), OP.mult)
                    gga.tensor_tensor(gt[:], gt[:], gt2[:], OP.add)
                    gga.tensor_tensor(gt2[:], u2, bc3(vs[cc][2]), OP.mult)
                    gga.tensor_tensor(Gc, gt[:], gt2[:], OP.add)
                nc.vector.tensor_scalar_mul(G[:], G[:], -2.0)

                # --------- e_r = sum_c Gt[c*3+r] (bcast over j) * PC_c
                def Gb(cc, r):
                    # G plane (a*3+b) holds (U M)_{b,a}; e_r needs (U M)_{cc,r}
                    return _ap(G, (r * 3 + cc) * F, [[0, J], [1, F]])

                for r in range(3):
                    er = epool.tile([P, JF], bf16, tag="e", name="er", bufs=2)
                    Ei("emult", r * 3 + 0).tensor_tensor(
                        er[:], cblk(PC, 0), Gb(0, r), OP.mult)
                    tb1 = oring.tile([P, JF], bf16, tag="O", name="eb", bufs=1)
                    Ei("emult", r * 3 + 1).tensor_tensor(
                        tb1[:], cblk(PC, 1), Gb(1, r), OP.mult)
                    Ei("eadd", r * 2).tensor_tensor(
                        er[:], er[:], tb1[:], OP.add)
                    tb2 = oring.tile([P, JF], bf16, tag="O", name="eb2", bufs=1)
                    Ei("emult", r * 3 + 2).tensor_tensor(
                        tb2[:], cblk(PC, 2), Gb(2, r), OP.mult)
                    Ei("eadd", r * 2 + 1).tensor_tensor(
                        er[:], er[:], tb2[:], OP.add)
                    # e_r *= TC_r (in place), then d2 += e_r
                    Ei("tcmul", r).tensor_tensor(
                        er[:], er[:], cblk(TC, r), OP.mult)
                    Ei("d2add", r).tensor_tensor(d2[:], d2[:], er[:], OP.add)

                if iters == 1:
                    nc._dbg.update(dict(
                        mean_p=mean_p[:], mean_t=mean_t[:], PC=PC[:], TC=TC[:],
                        P2=P2[:], T2=T2[:], H=H[:], d2=d2[:], G=G[:],
                        pn=pn[:], tn=tn[:], sS=sS[:], s2=s2[:],
                        lam0=lam0[:], lam1=lam1[:],
                        v0x=v0[0][:], v0y=v0[1][:], v0z=v0[2][:],
                        v1x=v1[0][:], v1y=v1[1][:], v1z=v1[2][:],
                        v2x=v2[0][:], v2y=v2[1][:], v2z=v2[2][:],
                        u=ub[:], rs0=rsig[0][:], rs1=rsig[1][:],
                    ))
                # --------- dist = sqrt(relu(d2)); sum over j; accumulate
                dr = sqp.tile([P, JF], bf16, tag="sq", name="dr", bufs=2)
                nc.scalar.activation(dr[:], d2[:], AF.Relu)
                nc.scalar.activation(dr[:], dr[:], AF.Sqrt)
                dsum = thinE_t()
                ntree(dr, dsum)
                nc.vector.tensor_tensor(acc[:], acc[:], dsum[:], OP.add)

            if iters == 1:
                body()
            else:
                with tc.For_i(0, iters, 1):
                    body()

            accs = persist.tile([P, 1], f32, tag="accs", name="accs")
            nc.vector.tensor_reduce(accs[:], acc[:], axis=AX.X, op=OP.add)
            nc.sync.dma_start(out_d[:], accs[:])

    nc.compile()
    return nc


_nc_cache = None


def get_nc():
    global _nc_cache
    if _nc_cache is None:
        _nc_cache = build_nc()
    return _nc_cache


def run(nc, pred, target, trace=False, **kw):
    pred2 = np.ascontiguousarray(np.asarray(pred), np.float32).reshape(B, JC)
    targ2 = np.ascontiguousarray(np.asarray(target), np.float32).reshape(B, JC)
    in_maps = [
        {"pred": pred2[c * BC:(c + 1) * BC], "target": targ2[c * BC:(c + 1) * BC]}
        for c in range(NCORES)
    ]
    res = run_bass_kernel_spmd(nc, in_maps, list(range(NCORES)), trace=trace, **kw)
    total = sum(r["partial"].astype(np.float64).sum() for r in res.results)
    loss = np.float32(total / (B * J))
    return loss, res


def kernel(pred, target):
    loss, _ = run(get_nc(), pred, target)
    return loss


# revision 17
# speedup vs baseline: 1.4899x; 1.0167x over previous
"""Batched Procrustes-alignment loss on 8 Trainium2 NeuronCores.

Data-parallel over batch (B=262144 -> 32768/core), laid out as [128
partitions, F=256] planes (one scalar per batch element per plane).

Per batch element:
  center pred/target over J=17 joints; per-joint squared norms P2/T2;
  scale s = tn/(pn+eps); H = PC^T TC via streamed per-plane products +
  j-tree reductions; closed-form symmetric 3x3 eigensolver on A = H^T H
  (trigonometric eigenvalues via acos/cos expressed with Arctan+Sin
  activations, eigenvectors via cross-of-rows, v2 = v0 x v1 so det(V)=+1
  and all LAPACK sign bookkeeping cancels); u_i = H v_i / sigma_i,
  u2 = (u0 x u1)/s; G = sum_i u_i (x) m_i scaled by -2s.
  dist_j^2 = s^2 P2_j - 2 s W_j + T2_j with W via rotated-pred planes
  e_r = sum_c Gt_cr PC_c, then d2 += TC_r*e_r; loss = mean sqrt.

Output: per-core per-partition partial sums [128,1]; host sums in float64
and divides by B*J.
"""
import numpy as np
import concourse.bass as bass
import concourse.mybir as mybir
import concourse.tile as tile
from concourse import bacc
from concourse.bass_utils import run_bass_kernel_spmd

AF = mybir.ActivationFunctionType
OP = mybir.AluOpType
AX = mybir.AxisListType
f32 = mybir.dt.float32
bf16 = mybir.dt.bfloat16

B, J, C = 262144, 17, 3
JC = J * C
NCORES = 8
BC = B // NCORES
P = 128
F = 256
JF = J * F
SUB = 64
NSUB = F // SUB
EPS = 1e-8
TINY = 1e-20

# engine assignment knobs ("v" = DVE vector, "g" = gpsimd Pool, "s" = scalar/Act)
KNOBS = dict(
    center=["g"] * 6,   # per (tensor*3 + c)
    omult=["v"] * 9,
    emult=["v", "g", "g", "v", "g", "g", "v", "g", "g"],
    eadd=["v"] * 6,
    tcmul=["g", "v", "v"],
    d2add=["v"] * 3,
    uassm="v",
    gassm="g",
    htree="v",
    sq="s",
)


def _ap(t, off, dims):
    a = t[:]
    return bass.AP(a.tensor, a.offset + off, [a.ap[0]] + dims)


def _pl(t, off, n):
    return _ap(t, off, [[1, n]])


def build_nc(iters=1, knobs=None):
    kn = dict(KNOBS)
    if knobs:
        kn.update(knobs)

    nc = bacc.Bacc("TRN2", target_bir_lowering=False)
    nc._dbg = {}
    pred_d = nc.dram_tensor("pred", [BC, JC], f32, kind="ExternalInput")
    targ_d = nc.dram_tensor("target", [BC, JC], f32, kind="ExternalInput")
    out_d = nc.dram_tensor("partial", [P, 1], f32, kind="ExternalOutput")

    def E(key):
        v = kn[key] if isinstance(kn[key], str) else None
        assert v is not None
        return {"v": nc.vector, "g": nc.gpsimd, "s": nc.scalar}[v]

    def Ei(key, i):
        return {"v": nc.vector, "g": nc.gpsimd, "s": nc.scalar}[kn[key][i]]

    with tile.TileContext(nc) as tc:
        with (
            tc.tile_pool(name="persist", bufs=1) as persist,
            tc.tile_pool(name="rawp", bufs=1) as rawp,
            tc.tile_pool(name="meanp", bufs=1) as meanp,
            tc.tile_pool(name="pctc", bufs=1) as pctcp,
            tc.tile_pool(name="oring", bufs=1) as oring,
            tc.tile_pool(name="sqp", bufs=1) as sqp,
            tc.tile_pool(name="hp", bufs=1) as hp,
            tc.tile_pool(name="ep", bufs=1) as epool,
            tc.tile_pool(name="late", bufs=1) as late,
            tc.tile_pool(name="thinE", bufs=1) as thinE,
            tc.tile_pool(name="psth", bufs=1, space="PSUM") as psth,
        ):
            acc = persist.tile([P, F], f32, tag="acc", name="acc")
            b2p3 = persist.tile([P, 1], f32, tag="b2p3", name="b2p3")
            b4p3 = persist.tile([P, 1], f32, tag="b4p3", name="b4p3")
            nc.gpsimd.memset(acc[:], 0.0)
            nc.gpsimd.memset(b2p3[:], 2.0943951023931953)
            nc.gpsimd.memset(b4p3[:], 1.0471975511965976)  # pi/3

            def thinE_t():
                return thinE.tile([P, F], f32, tag="te", name="te", bufs=12)

            _ps = {"n": 0, "banks": []}

            def psum_t(tg):
                i = _ps["n"]
                _ps["n"] += 1
                assert i < 16
                if i % 2 == 0:
                    _ps["banks"].append(
                        psth.tile([P, 2 * F], f32, tag=f"pb{i // 2}",
                                  name=f"pb{i // 2}"))
                blk = _ps["banks"][i // 2]
                off = (i % 2) * F

                class _T:
                    def __getitem__(self, _):
                        return _pl(blk, off, F)
                return _T()

            def body():
                _ps["n"] = 0
                _ps["banks"] = []
                # --------- tiles (allocated per iteration; tags reuse slots)
                PC = pctcp.tile([P, 3 * JF], bf16, tag="PC", name="PC")
                TC = pctcp.tile([P, 3 * JF], bf16, tag="TC", name="TC")
                P2 = sqp.tile([P, JF], bf16, tag="P2", name="P2")
                T2 = sqp.tile([P, JF], bf16, tag="T2", name="T2")
                H = hp.tile([P, 9 * F], f32, tag="H", name="H")
                d2 = late.tile([P, JF], f32, tag="d2", name="d2")
                G = late.tile([P, 9 * F], bf16, tag="G", name="G")
                mtmp = meanp.tile([P, 2880], f32, tag="mt", name="mtmp")
                mean_p = meanp.tile([P, 768], f32, tag="mp", name="mean_p")
                mean_t = meanp.tile([P, 768], f32, tag="mq", name="mean_t")

                # --------- per-sub-block load + mean tree + center
                for ti, (dram, mean, ctr) in enumerate(
                        ((pred_d, mean_p, PC), (targ_d, mean_t, TC))):
                    for s in range(NSUB):
                        raw = rawp.tile([P, JC * SUB], f32, tag="raw",
                                        name="raw", bufs=2)
                        off = (s * SUB) * JC
                        nc.sync.dma_start(
                            raw[:], bass.AP(dram[:].tensor, off,
                                            [[F * JC, P], [1, JC * SUB]]))
                        # mean tree over j (all 3 c at once); u = j*3+c
                        r1, r2, r3, r4 = 0, 1536, 2304, 2688
                        nc.vector.tensor_tensor(
                            _ap(mtmp, r1, [[24, SUB], [1, 24]]),
                            _ap(raw, 0, [[JC, SUB], [1, 24]]),
                            _ap(raw, 24, [[JC, SUB], [1, 24]]), OP.add)
                        nc.vector.tensor_tensor(
                            _ap(mtmp, r2, [[12, SUB], [1, 12]]),
                            _ap(mtmp, r1, [[24, SUB], [1, 12]]),
                            _ap(mtmp, r1 + 12, [[24, SUB], [1, 12]]), OP.add)
                        nc.vector.tensor_tensor(
                            _ap(mtmp, r3, [[6, SUB], [1, 6]]),
                            _ap(mtmp, r2, [[12, SUB], [1, 6]]),
                            _ap(mtmp, r2 + 6, [[12, SUB], [1, 6]]), OP.add)
                        nc.vector.tensor_tensor(
                            _ap(mtmp, r4, [[3, SUB], [1, 3]]),
                            _ap(mtmp, r3, [[6, SUB], [1, 3]]),
                            _ap(mtmp, r3 + 3, [[6, SUB], [1, 3]]), OP.add)
                        nc.vector.tensor_tensor(
                            _ap(mean, s * SUB * 3, [[3, SUB], [1, 3]]),
                            _ap(mtmp, r4, [[3, SUB], [1, 3]]),
                            _ap(raw, 48, [[JC, SUB], [1, 3]]), OP.add)
                        nc.vector.tensor_scalar_mul(
                            _ap(mean, s * SUB * 3, [[3, SUB], [1, 3]]),
                            _ap(mean, s * SUB * 3, [[3, SUB], [1, 3]]), 1.0 / J)
                        # center: PC_c[j, f] = raw - mean, per c
                        for c in range(3):
                            Ei("center", ti * 3 + c).tensor_tensor(
                                _ap(ctr, c * JF + s * SUB, [[F, J], [1, SUB]]),
                                _ap(raw, c, [[3, J], [JC, SUB]]),
                                _ap(mean, s * SUB * 3 + c, [[0, J], [3, SUB]]),
                                OP.subtract)

                # long-lived thin planes carved out of the (dead) raw slots
                tb1 = rawp.tile([P, JC * SUB], f32, tag="raw", name="tb1", bufs=2)
                tb2 = rawp.tile([P, JC * SUB], f32, tag="raw", name="tb2", bufs=2)
                nb = {"n": 0}

                def named(tg):
                    i = nb["n"]
                    nb["n"] += 1
                    assert i < 24
                    blk = tb1 if i < 12 else tb2
                    off = (i % 12) * F

                    class _T:
                        def __getitem__(self, _):
                            return _pl(blk, off, F)
                    return _T()

                def cblk(t, c):
                    return _pl(t, c * JF, JF)

                # --------- squares -> P2/T2 (Act) + adds (DVE)
                sq1 = sqp.tile([P, JF], bf16, tag="sq", name="sq1", bufs=2)
                nc.scalar.activation(P2[:], cblk(PC, 0), AF.Square)
                nc.scalar.activation(sq1[:], cblk(PC, 1), AF.Square)
                nc.vector.tensor_tensor(P2[:], P2[:], sq1[:], OP.add)
                sq2 = sqp.tile([P, JF], bf16, tag="sq", name="sq2", bufs=2)
                nc.scalar.activation(sq2[:], cblk(PC, 2), AF.Square)
                nc.vector.tensor_tensor(P2[:], P2[:], sq2[:], OP.add)
                nc.scalar.activation(T2[:], cblk(TC, 0), AF.Square)
                sq3 = sqp.tile([P, JF], bf16, tag="sq", name="sq3", bufs=2)
                nc.scalar.activation(sq3[:], cblk(TC, 1), AF.Square)
                nc.vector.tensor_tensor(T2[:], T2[:], sq3[:], OP.add)
                sq4 = sqp.tile([P, JF], bf16, tag="sq", name="sq4", bufs=2)
                nc.scalar.activation(sq4[:], cblk(TC, 2), AF.Square)
                nc.vector.tensor_tensor(T2[:], T2[:], sq4[:], OP.add)
                # sqrt planes for norms
                sp2 = sqp.tile([P, JF], bf16, tag="sq", name="sp2", bufs=2)
                nc.scalar.activation(sp2[:], P2[:], AF.Sqrt)
                st2 = sqp.tile([P, JF], bf16, tag="sq", name="st2", bufs=2)
                nc.scalar.activation(st2[:], T2[:], AF.Sqrt)

                # --------- O products (streamed) + H j-trees
                # H plane (c*3+r) = sum_j PC_c[j] * TC_r[j]
                for cc in range(3):
                    for r in range(3):
                        h = cc * 3 + r
                        O = oring.tile([P, JF], bf16, tag="O", name="O", bufs=1)
                        Ei("omult", h).tensor_tensor(
                            O[:], cblk(PC, cc), cblk(TC, r), OP.mult)
                        ht = hp.tile([P, 8 * F], f32, tag="ht", name="ht", bufs=1)
                        eng = E("htree")
                        eng.tensor_tensor(
                            ht[:], _ap(O, 0, [[F, 8], [1, F]]),
                            _ap(O, 8 * F, [[F, 8], [1, F]]), OP.add)
                        eng.tensor_tensor(
                            _pl(ht, 0, 4 * F), _pl(ht, 0, 4 * F),
                            _pl(ht, 4 * F, 4 * F), OP.add)
                        eng.tensor_tensor(
                            _pl(ht, 0, 2 * F), _pl(ht, 0, 2 * F),
                            _pl(ht, 2 * F, 2 * F), OP.add)
                        eng.tensor_tensor(
                            _pl(ht, 0, F), _pl(ht, 0, F), _pl(ht, F, F), OP.add)
                        eng.tensor_tensor(
                            _pl(H, h * F, F), _pl(ht, 0, F),
                            _pl(O, 16 * F, F), OP.add)

                def Hp(r, cc):
                    return _pl(H, (cc * 3 + r) * F, F)

                # --------- norm trees (pn from sp2, tn from st2)
                def ntree(srcpl, out):
                    ht = hp.tile([P, 8 * F], f32, tag="ht", name="nt", bufs=1)
                    nc.vector.tensor_tensor(
                        ht[:], _ap(srcpl, 0, [[F, 8], [1, F]]),
                        _ap(srcpl, 8 * F, [[F, 8], [1, F]]), OP.add)
                    nc.vector.tensor_tensor(
                        _pl(ht, 0, 4 * F), _pl(ht, 0, 4 * F),
                        _pl(ht, 4 * F, 4 * F), OP.add)
                    nc.vector.tensor_tensor(
                        _pl(ht, 0, 2 * F), _pl(ht, 0, 2 * F),
                        _pl(ht, 2 * F, 2 * F), OP.add)
                    nc.vector.tensor_tensor(
                        _pl(ht, 0, F), _pl(ht, 0, F), _pl(ht, F, F), OP.add)
                    nc.vector.tensor_tensor(
                        out[:], _pl(ht, 0, F), _pl(srcpl, 16 * F, F), OP.add)

                # --------- A = H^T H (6 upper entries), thin
                A6 = {}
                for (a, b) in ((0, 0), (0, 1), (0, 2), (1, 1), (1, 2), (2, 2)):
                    t1 = thinE_t()
                    nc.vector.tensor_tensor(t1[:], Hp(a, 0), Hp(b, 0), OP.mult)
                    t2 = thinE_t()
                    nc.vector.tensor_tensor(t2[:], Hp(a, 1), Hp(b, 1), OP.mult)
                    nc.vector.tensor_tensor(t1[:], t1[:], t2[:], OP.add)
                    t3 = thinE_t()
                    nc.vector.tensor_tensor(t3[:], Hp(a, 2), Hp(b, 2), OP.mult)
                    At = named(f"A{a}{b}")
                    nc.vector.tensor_tensor(At[:], t1[:], t3[:], OP.add)
                    A6[(a, b)] = At
                a00, a01, a02 = A6[(0, 0)], A6[(0, 1)], A6[(0, 2)]
                a11, a12, a22 = A6[(1, 1)], A6[(1, 2)], A6[(2, 2)]

                # --------- eigenvalues (closed form)
                q3 = thinE_t()
                nc.vector.tensor_tensor(q3[:], a00[:], a11[:], OP.add)
                nc.vector.tensor_tensor(q3[:], q3[:], a22[:], OP.add)
                m01, g0, g1 = named("m01"), named("g0"), named("g1")
                g2 = thinE_t()
                nc.vector.tensor_tensor(m01[:], a01[:], a01[:], OP.mult)
                nc.vector.tensor_tensor(g0[:], a01[:], a12[:], OP.mult)
                nc.vector.tensor_tensor(g1[:], a01[:], a02[:], OP.mult)
                nc.vector.tensor_tensor(g2[:], a02[:], a12[:], OP.mult)
                m02 = thinE_t()
                nc.vector.tensor_tensor(m02[:], a02[:], a02[:], OP.mult)
                m12 = thinE_t()
                nc.vector.tensor_tensor(m12[:], a12[:], a12[:], OP.mult)
                p1 = thinE_t()
                nc.vector.tensor_tensor(p1[:], m01[:], m02[:], OP.add)
                nc.vector.tensor_tensor(p1[:], p1[:], m12[:], OP.add)
                q = named("q")
                nc.vector.tensor_scalar_mul(q[:], q3[:], 1.0 / 3)
                b00, b11, b22 = thinE_t(), thinE_t(), thinE_t()
                nc.vector.tensor_tensor(b00[:], a00[:], q[:], OP.subtract)
                nc.vector.tensor_tensor(b11[:], a11[:], q[:], OP.subtract)
                nc.vector.tensor_tensor(b22[:], a22[:], q[:], OP.subtract)
                p2s = thinE_t()
                nc.vector.tensor_tensor(p2s[:], b00[:], b00[:], OP.mult)
                tb = thinE_t()
                nc.vector.tensor_tensor(tb[:], b11[:], b11[:], OP.mult)
                nc.vector.tensor_tensor(p2s[:], p2s[:], tb[:], OP.add)
                nc.vector.tensor_tensor(tb[:], b22[:], b22[:], OP.mult)
                nc.vector.tensor_tensor(p2s[:], p2s[:], tb[:], OP.add)
                nc.vector.scalar_tensor_tensor(
                    p2s[:], p1[:], 2.0, p2s[:], OP.mult, OP.add)
                pA = named("pA")
                nc.scalar.activation(pA[:], p2s[:], AF.Sqrt, scale=1.0 / 6)
                # fill: detB terms (independent of pA)
                c0 = thinE_t()
                nc.vector.tensor_tensor(c0[:], b11[:], b22[:], OP.mult)
                nc.vector.tensor_tensor(c0[:], c0[:], m12[:], OP.subtract)
                c1 = thinE_t()
                nc.vector.tensor_tensor(c1[:], a01[:], b22[:], OP.mult)
                nc.vector.tensor_tensor(c1[:], c1[:], g2[:], OP.subtract)
                c2 = thinE_t()
                nc.vector.tensor_tensor(c2[:], b11[:], a02[:], OP.mult)
                nc.vector.tensor_tensor(c2[:], g0[:], c2[:], OP.subtract)
                detB = thinE_t()
                nc.vector.tensor_tensor(detB[:], b00[:], c0[:], OP.mult)
                tdb = thinE_t()
                nc.vector.tensor_tensor(tdb[:], a01[:], c1[:], OP.mult)
                nc.vector.tensor_tensor(detB[:], detB[:], tdb[:], OP.subtract)
                nc.vector.tensor_tensor(tdb[:], a02[:], c2[:], OP.mult)
                nc.vector.tensor_tensor(detB[:], detB[:], tdb[:], OP.add)
                pinv = thinE_t()
                nc.vector.tensor_scalar_add(pinv[:], pA[:], TINY)
                nc.vector.reciprocal_approx_fast(pinv[:], pinv[:])
                p3 = thinE_t()
                nc.vector.tensor_tensor(p3[:], pinv[:], pinv[:], OP.mult)
                nc.vector.tensor_tensor(p3[:], p3[:], pinv[:], OP.mult)
                rc = thinE_t()
                nc.vector.tensor_tensor(rc[:], detB[:], p3[:], OP.mult)
                nc.vector.tensor_scalar(rc[:], rc[:], 0.5, 1.0, OP.mult, OP.min)
                nc.vector.tensor_scalar_max(rc[:], rc[:], -1.0)
                rr = thinE_t()
                nc.vector.tensor_tensor(rr[:], rc[:], rc[:], OP.mult)
                wA = thinE_t()
                nc.scalar.activation(wA[:], rr[:], AF.Sqrt, bias=1.0, scale=-1.0)
                # fill: pn tree
                pn = psum_t("pn")
                ntree(sp2, pn)
                rat = thinE_t()
                nc.vector.tensor_scalar_add(rat[:], wA[:], 1e-10)
                nc.vector.reciprocal_approx_fast(rat[:], rat[:])
                nc.vector.tensor_tensor(rat[:], rc[:], rat[:], OP.mult)
                # atan with range reduction: |x|>1 -> sign(x)*pi/2 - atan(1/x)
                a1 = thinE_t()
                nc.vector.tensor_scalar(a1[:], rat[:], 1.0, -1.0, OP.min, OP.max)
                rat2 = thinE_t()
                nc.vector.tensor_tensor(rat2[:], rat[:], rat[:], OP.mult)
                rinv = thinE_t()
                nc.vector.tensor_scalar_add(rinv[:], rat2[:], TINY)
                nc.vector.reciprocal_approx_fast(rinv[:], rinv[:])
                nc.vector.tensor_tensor(rinv[:], rat[:], rinv[:], OP.mult)
                nc.vector.tensor_scalar(rinv[:], rinv[:], 1.0, -1.0, OP.min, OP.max)
                sg = thinE_t()
                nc.vector.tensor_scalar(sg[:], rat[:], 1e10, 1.0, OP.mult, OP.min)
                nc.vector.tensor_scalar_max(sg[:], sg[:], -1.0)
                at1 = thinE_t()
                nc.scalar.activation(at1[:], a1[:], AF.Arctan)
                at2 = thinE_t()
                nc.scalar.activation(at2[:], rinv[:], AF.Arctan)
                atb = thinE_t()
                nc.vector.scalar_tensor_tensor(
                    atb[:], sg[:], 1.5707963267948966, at2[:],
                    OP.mult, OP.subtract)
                m_ = thinE_t()
                nc.vector.tensor_scalar_add(m_[:], rat2[:], -1.0)
                nc.vector.tensor_scalar(m_[:], m_[:], 1e10, 1.0, OP.mult, OP.min)
                nc.vector.tensor_scalar_max(m_[:], m_[:], 0.0)
                atn = thinE_t()
                nc.vector.tensor_tensor(atn[:], atb[:], at1[:], OP.subtract)
                nc.vector.tensor_tensor(atn[:], atn[:], m_[:], OP.mult)
                nc.vector.tensor_tensor(atn[:], atn[:], at1[:], OP.add)
                # fill: tn tree
                tn = psum_t("tn")
                ntree(st2, tn)
                cs1 = psum_t("cs1")
                nc.scalar.activation(cs1[:], atn[:], AF.Sin,
                                     bias=b2p3[:], scale=-1.0 / 3)
                cs2 = psum_t("cs2")
                nc.scalar.activation(cs2[:], atn[:], AF.Sin,
                                     bias=b4p3[:], scale=-1.0 / 3)
                # fill: s, s2, P~2 = s^2*P2 into d2 (f32), then d2 += T2
                sS = named("sS")
                nc.vector.tensor_scalar_add(sS[:], pn[:], EPS)
                nc.vector.reciprocal_approx_fast(sS[:], sS[:])
                nc.vector.tensor_tensor(sS[:], sS[:], tn[:], OP.mult)
                s2 = psum_t("s2")
                nc.vector.tensor_tensor(s2[:], sS[:], sS[:], OP.mult)
                nc.vector.tensor_tensor(
                    d2[:], P2[:], _ap(s2, 0, [[0, J], [1, F]]), OP.mult)
                nc.vector.tensor_tensor(d2[:], d2[:], T2[:], OP.add)
                lam0, lam1 = psum_t("lam0"), psum_t("lam1")
                tp = thinE_t()
                nc.vector.tensor_tensor(tp[:], pA[:], cs1[:], OP.mult)
                nc.vector.scalar_tensor_tensor(
                    lam0[:], tp[:], 2.0, q[:], OP.mult, OP.add)
                lam2 = thinE_t()
                nc.vector.tensor_tensor(tp[:], pA[:], cs2[:], OP.mult)
                nc.vector.scalar_tensor_tensor(
                    lam2[:], tp[:], -2.0, q[:], OP.mult, OP.add)
                nc.vector.scalar_tensor_tensor(
                    lam1[:], q[:], 3.0, lam0[:], OP.mult, OP.subtract)
                nc.vector.tensor_tensor(lam1[:], lam1[:], lam2[:], OP.subtract)

                # --------- eigenvectors v0 (lam0), v1 (lam1); v2 = v0 x v1
                def eigvec(lam, pref):
                    vx = named(pref + "x")
                    vy = named(pref + "y")
                    vz = named(pref + "z")
                    b0 = thinE_t()
                    nc.vector.tensor_tensor(b0[:], a00[:], lam[:], OP.subtract)
                    b1 = thinE_t()
                    nc.vector.tensor_tensor(b1[:], a11[:], lam[:], OP.subtract)
                    nc.vector.tensor_tensor(vx[:], a02[:], b1[:], OP.mult)
                    nc.vector.tensor_tensor(vx[:], g0[:], vx[:], OP.subtract)
                    nc.vector.tensor_tensor(vy[:], b0[:], a12[:], OP.mult)
                    nc.vector.tensor_tensor(vy[:], g1[:], vy[:], OP.subtract)
                    nc.vector.tensor_tensor(vz[:], b0[:], b1[:], OP.mult)
                    nc.vector.tensor_tensor(vz[:], vz[:], m01[:], OP.subtract)
                    n2 = thinE_t()
                    nc.vector.tensor_tensor(n2[:], vx[:], vx[:], OP.mult)
                    t2_ = thinE_t()
                    nc.vector.tensor_tensor(t2_[:], vy[:], vy[:], OP.mult)
                    nc.vector.tensor_tensor(n2[:], n2[:], t2_[:], OP.add)
                    nc.vector.tensor_tensor(t2_[:], vz[:], vz[:], OP.mult)
                    nc.vector.tensor_tensor(n2[:], n2[:], t2_[:], OP.add)
                    ns = thinE_t()
                    nc.scalar.activation(ns[:], n2[:], AF.Sqrt)
                    nc.vector.tensor_scalar_add(ns[:], ns[:], TINY)
                    nc.vector.reciprocal_approx_fast(ns[:], ns[:])
                    nc.vector.tensor_tensor(vx[:], vx[:], ns[:], OP.mult)
                    nc.vector.tensor_tensor(vy[:], vy[:], ns[:], OP.mult)
                    nc.vector.tensor_tensor(vz[:], vz[:], ns[:], OP.mult)
                    return vx, vy, vz

                v0 = eigvec(lam0, "v0")
                v1 = eigvec(lam1, "v1")
                v2 = (named("v2x"), named("v2y"), named("v2z"))
                cr = ((1, 2), (2, 0), (0, 1))
                for r in range(3):
                    i1, i2 = cr[r]
                    t1 = thinE_t()
                    nc.vector.tensor_tensor(t1[:], v0[i1][:], v1[i2][:], OP.mult)
                    t2_ = thinE_t()
                    nc.vector.tensor_tensor(t2_[:], v0[i2][:], v1[i1][:], OP.mult)
                    nc.vector.tensor_tensor(v2[r][:], t1[:], t2_[:], OP.subtract)

                # --------- rsig_i = s / sigma_i ; u_i = H v_i * rsig_i
                rsig = []
                for i, lam in enumerate((lam0, lam1)):
                    rl = thinE_t()
                    nc.scalar.activation(rl[:], lam[:], AF.Relu)
                    sg = thinE_t()
                    nc.scalar.activation(sg[:], rl[:], AF.Sqrt)
                    nc.vector.tensor_scalar_add(sg[:], sg[:], TINY)
                    nc.vector.reciprocal_approx_fast(sg[:], sg[:])
                    rs = psum_t(f"rs{i}")
                    nc.vector.tensor_tensor(rs[:], sg[:], sS[:], OP.mult)
                    rsig.append(rs)

                ub = meanp.tile([P, 2880], f32, tag="mt", name="ublock")
                u0 = _ap(ub, 0, [[F, 3], [1, F]])
                u1 = _ap(ub, 3 * F, [[F, 3], [1, F]])
                u2 = _ap(ub, 6 * F, [[F, 3], [1, F]])

                def up(ui, r):
                    return _pl(ub, ui * 3 * F + r * F, F)

                def bc3(t):
                    return _ap(t, 0, [[0, 3], [1, F]])

                def HCg(k):
                    # H rows group for fixed k: planes (c*3+k)... careful:
                    # u_i[r] = sum_k H[r,k] v_i[k]; H[r,k] stored plane (r*3+k)?
                    # H plane (c*3+r) = H_cr = sum_j PC_c TC_r -> H[c,r].
                    # reference H_ik = sum_j pc_{j,i} tc_{j,k} -> H[i,k] = plane(i*3+k)
                    # u_i[r] = sum_k H[r,k] (v_i)_k: planes (r*3+k), r varies
                    # group for fixed k over r: offset k*F stride 3F
                    return _ap(H, k * F, [[3 * F, 3], [1, F]])

                uga = {"v": nc.vector, "g": nc.gpsimd}[kn["uassm"]]
                for i, (vv, rs) in enumerate(((v0, rsig[0]), (v1, rsig[1]))):
                    udst = (u0, u1)[i]
                    gt = meanp.tile([P, 768], f32, tag="mp", name="gt", bufs=1)
                    uga.tensor_tensor(udst, HCg(0), bc3(vv[0]), OP.mult)
                    uga.tensor_tensor(gt[:], HCg(1), bc3(vv[1]), OP.mult)
                    uga.tensor_tensor(udst, udst, gt[:], OP.add)
                    uga.tensor_tensor(gt[:], HCg(2), bc3(vv[2]), OP.mult)
                    uga.tensor_tensor(udst, udst, gt[:], OP.add)
                    uga.tensor_tensor(udst, udst, bc3(rs), OP.mult)
                # u2 = cross(u0, u1) / s
                invs = psum_t("invs")
                nc.vector.tensor_scalar_add(invs[:], sS[:], TINY)
                nc.vector.reciprocal_approx_fast(invs[:], invs[:])
                for r in range(3):
                    i1, i2 = cr[r]
                    t1 = thinE_t()
                    nc.vector.tensor_tensor(t1[:], up(0, i1), up(1, i2), OP.mult)
                    t2_ = thinE_t()
                    nc.vector.tensor_tensor(t2_[:], up(0, i2), up(1, i1), OP.mult)
                    nc.vector.tensor_tensor(t1[:], t1[:], t2_[:], OP.subtract)
                    nc.vector.tensor_tensor(up(2, r), t1[:], invs[:], OP.mult)

                # --------- G: plane (c*3+r) = sum_i u_i[r] * (v_c)_i, then *-2
                gga = {"v": nc.vector, "g": nc.gpsimd}[kn["gassm"]]
                vs = (v0, v1, v2)
                for cc in range(3):
                    Gc = _ap(G, cc * 3 * F, [[F, 3], [1, F]])
                    gt = meanp.tile([P, 768], f32, tag="mp", name="gt2", bufs=1)
                    gt2 = meanp.tile([P, 768], f32, tag="mq", name="gt3", bufs=1)
                    gga.tensor_tensor(gt[:], u0, bc3(vs[cc][0]), OP.mult)
                    gga.tensor_tensor(gt2[:], u1, bc3(vs[cc][1]), OP.mult)
                    gga.tensor_tensor(gt[:], gt[:], gt2[:], OP.add)
                    gga.tensor_tensor(gt2[:], u2, bc3(vs[cc][2]), OP.mult)
                    gga.tensor_tensor(Gc, gt[:], gt2[:], OP.add)
                nc.vector.tensor_scalar_mul(G[:], G[:], -2.0)

                # --------- e_r = sum_c Gt[c*3+r] (bcast over j) * PC_c
                def Gb(cc, r):
                    # G plane (a*3+b) holds (U M)_{b,a}; e_r needs (U M)_{cc,r}
                    return _ap(G, (r * 3 + cc) * F, [[0, J], [1, F]])

                for r in range(3):
                    er = epool.tile([P, JF], bf16, tag="e", name="er", bufs=2)
                    Ei("emult", r * 3 + 0).tensor_tensor(
                        er[:], cblk(PC, 0), Gb(0, r), OP.mult)
                    tb1 = oring.tile([P, JF], bf16, tag="O", name="eb", bufs=1)
                    Ei("emult", r * 3 + 1).tensor_tensor(
                        tb1[:], cblk(PC, 1), Gb(1, r), OP.mult)
                    Ei("eadd", r * 2).tensor_tensor(
                        er[:], er[:], tb1[:], OP.add)
                    tb2 = oring.tile([P, JF], bf16, tag="O", name="eb2", bufs=1)
                    Ei("emult", r * 3 + 2).tensor_tensor(
                        tb2[:], cblk(PC, 2), Gb(2, r), OP.mult)
                    Ei("eadd", r * 2 + 1).tensor_tensor(
                        er[:], er[:], tb2[:], OP.add)
                    # e_r *= TC_r (in place), then d2 += e_r
                    Ei("tcmul", r).tensor_tensor(
                        er[:], er[:], cblk(TC, r), OP.mult)
                    Ei("d2add", r).tensor_tensor(d2[:], d2[:], er[:], OP.add)

                if iters == 1:
                    nc._dbg.update(dict(
                        mean_p=mean_p[:], mean_t=mean_t[:], PC=PC[:], TC=TC[:],
                        P2=P2[:], T2=T2[:], H=H[:], d2=d2[:], G=G[:],
                        pn=pn[:], tn=tn[:], sS=sS[:], s2=s2[:],
                        lam0=lam0[:], lam1=lam1[:],
                        v0x=v0[0][:], v0y=v0[1][:], v0z=v0[2][:],
                        v1x=v1[0][:], v1y=v1[1][:], v1z=v1[2][:],
                        v2x=v2[0][:], v2y=v2[1][:], v2z=v2[2][:],
                        u=ub[:], rs0=rsig[0][:], rs1=rsig[1][:],
                    ))
                # --------- dist = sqrt(relu(d2)); sum over j; accumulate
                dr = sqp.tile([P, JF], bf16, tag="sq", name="dr", bufs=2)
                nc.scalar.activation(dr[:], d2[:], AF.Relu)
                nc.scalar.activation(dr[:], dr[:], AF.Sqrt)
                dsum = thinE_t()
                ntree(dr, dsum)
                nc.vector.tensor_tensor(acc[:], acc[:], dsum[:], OP.add)

            if iters == 1:
                body()
            else:
                with tc.For_i(0, iters, 1):
                    body()

            accs = persist.tile([P, 1], f32, tag="accs", name="accs")
            nc.vector.tensor_reduce(accs[:], acc[:], axis=AX.X, op=OP.add)
            nc.sync.dma_start(out_d[:], accs[:])

    nc.compile()
    return nc


_nc_cache = None


def get_nc():
    global _nc_cache
    if _nc_cache is None:
        _nc_cache = build_nc()
    return _nc_cache


def run(nc, pred, target, trace=False, **kw):
    pred2 = np.ascontiguousarray(np.asarray(pred), np.float32).reshape(B, JC)
    targ2 = np.ascontiguousarray(np.asarray(target), np.float32).reshape(B, JC)
    in_maps = [
        {"pred": pred2[c * BC:(c + 1) * BC], "target": targ2[c * BC:(c + 1) * BC]}
        for c in range(NCORES)
    ]
    res = run_bass_kernel_spmd(nc, in_maps, list(range(NCORES)), trace=trace, **kw)
    total = sum(r["partial"].astype(np.float64).sum() for r in res.results)
    loss = np.float32(total / (B * J))
    return loss, res


def kernel(pred, target):
    loss, _ = run(get_nc(), pred, target)
    return loss
